# revision 22
# baseline (speedup 1.0000x reference)
"""Trainium2 Bass kernel for nn_CascadeGradNetOURS (dense_mlp).

Math (reference):
    h = x @ W.T                       # (B, E), shared by all layers
    z = beta[0] * (h + b[0])
    for i in 0..6:
        z = beta[i+1]*(h + b[i+1]) + alpha[i]*relu(z)
    z = alpha[7] * relu(z)
    out = z @ W + bias_last           # (B, IN)

Device formulation (per core, batch-sharded 1024 rows, transposed layout
hT[e, b] so per-layer alpha/beta/bias become per-PARTITION scalars).

Baseline recurrence (sign-deferred, verified):
    Vt_1 = h;  U_k = relu(as_k*Vt_k + ab_k);  Vt_{k+1} = ss_k*h + U_k
    z = U_8 (|alpha_7| & sign folded into W2 rows); out = z @ W2 + blast.

Op grouping here avoids the 1x-rate SCALAR_TENSOR_TENSOR entirely:
all elementwise work is 2x-rate tensor_scalar (2 ALU slots, per-partition
AP scalars), 2x tensor_tensor adds batched over ec-pairs, plus one
ACTIVATE per layer for 6 of the 8 relus:
    D-layer 1:  P_1  = ts(h, as_1, ab_1; mult, add)
                Uo_1 = ts(P_1, 0.0, as_2; max, mult)
    D-layer 2:  hb_2 = ts(h, as_2*ss_1, ab_2; mult, add)
                P_2  = TT_add(hb_2, Uo_1)        [ec-pair batched]
                Uo_2 = ts(P_2, 0.0; max)         [ec-pair batched]
    A-layers k=3..8:
                hb_k = ts(h, ss_{k-1}; mult)
                Vt_k = TT_add(hb_k, U_{k-1})     [ec-pair batched]
                U_k  = ACT relu(as_k*Vt_k + ab_k)
Validated vs the fp64 oracle in numpy: rel err ~ 4.9e-4 of output absmax.
"""

import os

os.environ.setdefault("MYCRO_LOCAL_CACHE", "1")

import numpy as np

import concourse.bacc as bacc
import concourse.bass as bass
import concourse.mybir as mybir
from concourse.tile import TileContext

N_CORES = 8
B, IN, E, L = 8192, 1024, 4096, 8
BC = B // N_CORES          # 1024 batch rows per core
NI = IN // 128             # 8 i-chunks
NE = E // 128              # 32 e-chunks
F16 = mybir.dt.float16
F32 = mybir.dt.float32
NCONST = 24

GROUP = 4                  # e-chunks interleaved in the cascade pipeline
W1ECS = 20                 # mm2 window-1 depth (overlapped under the cascade)
EVICT_DVE_MOD = 16         # every Nth h-eviction runs on DVE, rest on ACT
GP_LAYERS = ()             # GPSIMD TT offload: net loss (shared SBUF port
                           # contention doubles DVE ts cost) — keep empty


_SEQ_ONLY = {
    "InstUnconditionalBranch",
    "InstCall",
    "InstISA",
}


def _legalize_waits(nc):
    """Datapath instructions carry exactly ONE semaphore wait slot in the
    64-byte ISA encoding (walrus errors on more). Engine sequencers execute
    their stream in order, so any extra waits can be hoisted onto single-wait
    NoOps inserted immediately before the capped instruction — semantically
    identical (all waits still complete before the instruction executes).
    For HWDGE DMAs prefer keeping a DMA-queue wait in-descriptor and hoist
    engine-sem waits to the sequencer."""
    import bass_rust

    uid = 0
    for bb in nc.m.functions[0].blocks:
        insts = bb.instructions  # live list
        newlist = []
        for i in insts:
            cls = i.__class__.__name__
            si = i.sync_info
            if cls in _SEQ_ONLY or si is None or len(si.on_wait) <= 1:
                newlist.append(i)
                continue
            waits = list(si.on_wait)
            if cls == "InstDMACopy":
                dmaw = [w for w in waits if w.ant_name.startswith("DMA")]
                keep = dmaw[-1] if dmaw else waits[-1]
            else:
                keep = waits[-1]
            rest = [w for w in waits if w is not keep]
            for w in rest:
                uid += 1
                nop = mybir.InstNoOp(
                    name=f"waitnop-{uid}-{i.name}",
                    engine=i.engine,
                    bass_nofuse=True,
                )
                nop.sync_info = bass_rust.SyncInfo(on_wait=[w], on_update=[])
                newlist.append(nop)
            si.on_wait = [keep]
            newlist.append(i)
        if len(newlist) != len(insts):
            insts[:] = newlist


def build_nc() -> bass.Bass:
    nc = bacc.Bacc()
    AL = mybir.AluOpType
    AF = mybir.ActivationFunctionType

    xTd = nc.declare_dram_parameter("xT", [128, NI, BC], F16, isOutput=False)
    WTd = nc.declare_dram_parameter("WT", [128, NE, NI, 128], F16, isOutput=False)
    W2d = nc.declare_dram_parameter("W2", [128, NE, IN], F16, isOutput=False)
    Cd = nc.declare_dram_parameter("consts", [128, NE, NCONST], F32, isOutput=False)
    Bd = nc.declare_dram_parameter("blast", [128, NI], F32, isOutput=False)
    Od = nc.declare_dram_parameter("outT", [128, NI, BC], F16, isOutput=True)

    with TileContext(nc) as tc:
        with (
            tc.tile_pool(name="persist", bufs=1) as persist,
            tc.tile_pool(name="wtp", bufs=4) as wtp,
            tc.tile_pool(name="w2p", bufs=3) as w2p,
            tc.tile_pool(name="hsbp", bufs=4) as hsbp,
            tc.tile_pool(name="upool", bufs=4) as upool,
            tc.tile_pool(name="vtpool", bufs=3) as vtpool,
            tc.tile_pool(name="hbpool", bufs=3) as hbpool,
            tc.tile_pool(name="ppool", bufs=2) as ppool,
            tc.tile_pool(name="outp", bufs=2) as outp,
            tc.tile_pool(name="psum_h", bufs=3, space="PSUM") as psum_h,
            tc.tile_pool(name="psum_o", bufs=2, space="PSUM") as psum_o,
        ):
            consts_sb = persist.tile([128, NE, NCONST], F32)
            nc.gpsimd.dma_start(out=consts_sb, in_=Cd[:, :, :])
            blast_sb = persist.tile([128, NI], F32)
            nc.gpsimd.dma_start(out=blast_sb, in_=Bd[:, :])
            x_sb = persist.tile([128, NI, BC], F16)
            # x split across two DMA queues with the first weight tile
            # leading the scalar queue, so mm1 can start within ~4us
            pre_wt = {}
            pre_wt[0] = wtp.tile([128, NI, 128], F16, tag="wt", name="wt_pre0")
            nc.scalar.dma_start(out=pre_wt[0], in_=WTd[:, 0, :, :])
            for i in range(NI):
                q = nc.sync if i % 2 == 0 else nc.scalar
                q.dma_start(out=x_sb[:, i, :], in_=xTd[:, i, :])
            for ec in range(1, GROUP):
                wt = wtp.tile([128, NI, 128], F16, tag="wt")
                nc.gpsimd.dma_start(out=wt, in_=WTd[:, ec, :, :])
                pre_wt[ec] = wt
            z_sb = persist.tile([128, NE, BC], F16)
            o_acc = persist.tile([128, NI, BC], F16)

            def c_ap(ec, col):
                return consts_sb[:, ec, col : col + 1]

            w2_tiles = {}

            def load_w2(ic, ec_lo, ec_hi):
                t = w2p.tile(
                    [128, W1ECS, 128], F16, tag="w2", name=f"w2_{ic}_{ec_lo}"
                )
                nc.sync.dma_start(
                    out=t[:, : ec_hi - ec_lo, :],
                    in_=W2d[:, ec_lo:ec_hi, ic * 128 : (ic + 1) * 128],
                )
                w2_tiles[(ic, ec_lo)] = t

            def emit_mm2_group(ic, hf, ec_lo, ec_hi, into_acc):
                w2t = w2_tiles[(ic, ec_lo)]
                ops = psum_o.tile(
                    [128, 512], F32, tag="o", name=f"o_{ic}_{hf}_{ec_lo}"
                )
                for ec in range(ec_lo, ec_hi):
                    nc.tensor.matmul(
                        ops,
                        w2t[:, ec - ec_lo, :],
                        z_sb[:, ec, hf * 512 : (hf + 1) * 512],
                        start=(ec == ec_lo),
                        stop=(ec == ec_hi - 1),
                    )
                bsl = hf * 512
                if into_acc:
                    # bias_last folded here; fp16 partial staging
                    nc.scalar.activation(
                        out=o_acc[:, ic, bsl : bsl + 512],
                        in_=ops,
                        func=AF.Identity,
                        bias=blast_sb[:, ic : ic + 1],
                        scale=1.0,
                    )
                else:
                    osb = outp.tile([128, 512], F16, tag="osb")
                    nc.vector.tensor_tensor(
                        out=osb, in0=o_acc[:, ic, bsl : bsl + 512], in1=ops,
                        op=AL.add,
                    )
                    oq = (nc.sync, nc.scalar, nc.gpsimd)[(2 * ic + hf) % 3]
                    oq.dma_start(out=Od[:, ic, bsl : bsl + 512], in_=osb)

            # window-1 whole-ic units spread over phase-A tail groups,
            # emitted AFTER each group's mm1 so they don't starve the cascade
            w1_sched = {20: range(0, 3), 24: range(3, 5), 28: range(5, 7)}

            # ---------------- Phase A: mm1 + cascade ----------------
            for g0 in range(0, NE, GROUP):
                ecs = list(range(g0, g0 + GROUP))
                h_ps = {}
                for ec in ecs:
                    if ec in pre_wt:
                        wt = pre_wt[ec]
                    else:
                        wt = wtp.tile([128, NI, 128], F16, tag="wt")
                        nc.sync.dma_start(out=wt, in_=WTd[:, ec, :, :])
                    hp = psum_h.tile([128, BC], F32, tag="h")
                    for i in range(NI):
                        lhsT = wt[:, i, :]
                        for hf in range(2):
                            nc.tensor.matmul(
                                hp[:, hf * 512 : (hf + 1) * 512],
                                lhsT,
                                x_sb[:, i, hf * 512 : (hf + 1) * 512],
                                start=(i == 0),
                                stop=(i == NI - 1),
                            )
                    h_ps[ec] = hp

                for ic in w1_sched.get(g0, ()):
                    load_w2(ic, 0, W1ECS)
                    for hf in range(2):
                        emit_mm2_group(ic, hf, 0, W1ECS, into_acc=True)

                pairs = [(ecs[0], ecs[1]), (ecs[2], ecs[3])]
                hsb = {}
                for pi, (ea, eb) in enumerate(pairs):
                    t = hsbp.tile([128, 2, BC], F16, tag="hsb", name=f"hsb_{ea}")
                    for j, ec in ((0, ea), (1, eb)):
                        if ec % EVICT_DVE_MOD == 0:
                            nc.vector.tensor_copy(out=t[:, j, :], in_=h_ps[ec])
                        else:
                            nc.scalar.copy(out=t[:, j, :], in_=h_ps[ec])
                    hsb[pi] = t

                # ---- cascade, layer-major across the 2 pairs ----
                u_cur = {}
                # D-layer 1: P_1 then Uo_1 = relu(P_1) * as_2
                for pi, (ea, eb) in enumerate(pairs):
                    pt = ppool.tile([128, 2, BC], F16, tag="p", name=f"p1_{ea}")
                    for j, ec in ((0, ea), (1, eb)):
                        nc.vector.tensor_scalar(
                            pt[:, j, :], hsb[pi][:, j, :],
                            c_ap(ec, 8), c_ap(ec, 16), AL.mult, AL.add,
                        )
                    ut = upool.tile([128, 2, BC], F16, tag="u", name=f"u1_{ea}")
                    for j, ec in ((0, ea), (1, eb)):
                        nc.vector.tensor_scalar(
                            ut[:, j, :], pt[:, j, :],
                            0.0, c_ap(ec, 9), AL.max, AL.mult,
                        )
                    u_cur[pi] = ut
                # D-layer 2: hb_2, P_2 = hb_2 + Uo_1, Uo_2 = relu(P_2)
                for pi, (ea, eb) in enumerate(pairs):
                    hbt = hbpool.tile([128, 2, BC], F16, tag="hb", name=f"hb2_{ea}")
                    for j, ec in ((0, ea), (1, eb)):
                        nc.vector.tensor_scalar(
                            hbt[:, j, :], hsb[pi][:, j, :],
                            c_ap(ec, 6), c_ap(ec, 17), AL.mult, AL.add,
                        )
                    pt = ppool.tile([128, 2, BC], F16, tag="p", name=f"p2_{ea}")
                    nc.vector.tensor_tensor(
                        out=pt[:, :, :], in0=hbt[:, :, :], in1=u_cur[pi][:, :, :],
                        op=AL.add,
                    )
                    ut = upool.tile([128, 2, BC], F16, tag="u", name=f"u2_{ea}")
                    nc.vector.tensor_scalar(
                        ut[:, :, :], pt[:, :, :], 0.0, None, AL.max,
                    )
                    u_cur[pi] = ut
                # A-layers 3..8
                for k in range(3, L + 1):
                    for pi, (ea, eb) in enumerate(pairs):
                        hbt = hbpool.tile(
                            [128, 2, BC], F16, tag="hb", name=f"hb{k}_{ea}"
                        )
                        for j, ec in ((0, ea), (1, eb)):
                            nc.vector.tensor_scalar(
                                hbt[:, j, :], hsb[pi][:, j, :],
                                c_ap(ec, k - 3), None, AL.mult,
                            )
                        vt = vtpool.tile(
                            [128, 2, BC], F16, tag="vt", name=f"vt{k}_{ea}"
                        )
                        tt_eng = nc.gpsimd if k in GP_LAYERS else nc.vector
                        tt_eng.tensor_tensor(
                            out=vt[:, :, :], in0=hbt[:, :, :],
                            in1=u_cur[pi][:, :, :], op=AL.add,
                        )
                        if k < L:
                            ut = upool.tile(
                                [128, 2, BC], F16, tag="u", name=f"u{k}_{ea}"
                            )
                        for j, ec in ((0, ea), (1, eb)):
                            dst = z_sb[:, ec, :] if k == L else ut[:, j, :]
                            nc.scalar.activation(
                                out=dst,
                                in_=vt[:, j, :],
                                func=AF.Relu,
                                bias=c_ap(ec, 16 + k - 1),
                                scale=c_ap(ec, 8 + k - 1),
                            )
                        if k < L:
                            u_cur[pi] = ut

            # late window-1 unit fills the PE gap while the last cascade
            # group finishes
            load_w2(7, 0, W1ECS)
            for hf in range(2):
                emit_mm2_group(7, hf, 0, W1ECS, into_acc=True)

            # ---------------- Phase B: mm2 window-2 + combine ----------------
            for ic in range(NI):
                load_w2(ic, W1ECS, NE)
                for hf in range(2):
                    emit_mm2_group(ic, hf, W1ECS, NE, into_acc=False)

    nc.compile()
    return nc


def _prep_inputs(x, W, biases, bias_last, alpha, beta):
    """Host-side shard/relayout/constant precompute. Returns per-core in_maps."""
    x = np.asarray(x, np.float32)
    W = np.asarray(W, np.float32)
    biases = np.asarray(biases, np.float32)
    bias_last = np.asarray(bias_last, np.float32)
    alpha = np.asarray(alpha, np.float32)
    beta = np.asarray(beta, np.float32)

    sgn = lambda a: np.where(a >= 0, 1.0, -1.0).astype(np.float32)
    Bk = beta * biases[:L]                      # (8, E)
    sigma = np.ones((L + 1, E), np.float32)     # sigma[k], k=1..8
    for k in range(1, L):
        sigma[k + 1] = sgn(alpha[k - 1])
    s_last = sgn(alpha[L - 1])

    ss = np.zeros((L + 1, E), np.float32)       # ss_k, k=1..7
    as_ = np.zeros((L + 1, E), np.float32)      # as_k, k=1..8
    ab = np.zeros((L + 1, E), np.float32)
    for k in range(1, L):
        ss[k] = sigma[k + 1] * beta[k]
    for k in range(1, L + 1):
        sck = sigma[k] * np.abs(alpha[k - 1])
        if k == 1:
            sck = sck * beta[0]                 # layer-1 reads h directly
        as_[k] = sck
        ab[k] = np.abs(alpha[k - 1]) * Bk[k - 1]

    consts = np.zeros((E, NCONST), np.float32)
    for k in range(2, L):                       # ss_2..ss_7 -> cols 0..5
        consts[:, k - 2] = ss[k]
    consts[:, 6] = as_[2] * ss[1]               # hb_2 scale
    for k in range(1, L + 1):                   # as cols 8..15, ab cols 16..23
        consts[:, 7 + k] = as_[k]
        consts[:, 15 + k] = ab[k]
    consts_t = np.ascontiguousarray(
        consts.reshape(NE, 128, NCONST).transpose(1, 0, 2)
    )

    WT_t = np.ascontiguousarray(
        W.T.reshape(NI, 128, NE, 128).transpose(1, 2, 0, 3).astype(np.float16)
    )
    W2 = W * s_last[:, None]
    W2_t = np.ascontiguousarray(
        W2.reshape(NE, 128, IN).transpose(1, 0, 2).astype(np.float16)
    )
    blast_t = np.ascontiguousarray(bias_last.reshape(NI, 128).T)

    in_maps = []
    for c in range(N_CORES):
        xc = x[c * BC : (c + 1) * BC]           # (BC, IN)
        xT = np.ascontiguousarray(
            xc.T.reshape(NI, 128, BC).transpose(1, 0, 2).astype(np.float16)
        )
        in_maps.append(
            {
                "xT": xT,
                "WT": WT_t,
                "W2": W2_t,
                "consts": consts_t,
                "blast": blast_t,
            }
        )
    return in_maps


_NC_CACHE = None


def _install_ntff_hook():
    """The agent image's antenv lacks axon_hooks; rebuild it from the boot
    helper so run_bass_kernel_spmd(trace=True) can capture NTFF profiles."""
    import sys
    import types

    if "antenv.axon_hooks" in sys.modules:
        return
    try:
        from trn_agent_boot.trn_boot import _ntff_profile_via_ctypes

        hook = _ntff_profile_via_ctypes("/opt/axon/libaxon_pjrt.so")
    except Exception:
        hook = None
    m = types.ModuleType("antenv.axon_hooks")
    m.get_axon_ntff_profile_hook = lambda: hook
    m.set_axon_ntff_profile_hook = lambda h: None
    sys.modules["antenv.axon_hooks"] = m


def run(inputs: dict, trace: bool = False):
    """Returns (out, BassKernelResults)."""
    global _NC_CACHE
    from concourse.bass_utils import run_bass_kernel_spmd

    if trace:
        _install_ntff_hook()

    if _NC_CACHE is None:
        _NC_CACHE = build_nc()
    nc = _NC_CACHE
    in_maps = _prep_inputs(**inputs)
    res = run_bass_kernel_spmd(nc, in_maps, list(range(N_CORES)), trace=trace)
    out = np.empty((B, IN), np.float32)
    for c in range(N_CORES):
        oc = np.asarray(res.results[c]["outT"]).astype(np.float32)
        out_core = oc.transpose(1, 0, 2).reshape(IN, BC) # (IN, BC) = outT
        out[c * BC : (c + 1) * BC] = out_core.T
    return out, res


def kernel(x, W, biases, bias_last, alpha, beta) -> np.ndarray:
    out, _ = run(
        dict(x=x, W=W, biases=biases, bias_last=bias_last, alpha=alpha, beta=beta)
    )
    return out


# revision 23
# speedup vs baseline: 1.0275x; 1.0275x over previous
"""Trainium2 Bass kernel for nn_CascadeGradNetOURS (dense_mlp).

Math (reference):
    h = x @ W.T                       # (B, E), shared by all layers
    z = beta[0] * (h + b[0])
    for i in 0..6:
        z = beta[i+1]*(h + b[i+1]) + alpha[i]*relu(z)
    z = alpha[7] * relu(z)
    out = z @ W + bias_last           # (B, IN)

Device formulation (per core, batch-sharded 1024 rows, transposed layout
hT[e, b] so per-layer alpha/beta/bias become per-PARTITION scalars).

Baseline recurrence (sign-deferred, verified):
    Vt_1 = h;  U_k = relu(as_k*Vt_k + ab_k);  Vt_{k+1} = ss_k*h + U_k
    z = U_8 (|alpha_7| & sign folded into W2 rows); out = z @ W2 + blast.

Op grouping here avoids the 1x-rate SCALAR_TENSOR_TENSOR entirely:
all elementwise work is 2x-rate tensor_scalar (2 ALU slots, per-partition
AP scalars), 2x tensor_tensor adds batched over ec-pairs, plus one
ACTIVATE per layer for 6 of the 8 relus:
    D-layer 1:  P_1  = ts(h, as_1, ab_1; mult, add)
                Uo_1 = ts(P_1, 0.0, as_2; max, mult)
    D-layer 2:  hb_2 = ts(h, as_2*ss_1, ab_2; mult, add)
                P_2  = TT_add(hb_2, Uo_1)        [ec-pair batched]
                Uo_2 = ts(P_2, 0.0; max)         [ec-pair batched]
    A-layers k=3..8:
                hb_k = ts(h, ss_{k-1}; mult)
                Vt_k = TT_add(hb_k, U_{k-1})     [ec-pair batched]
                U_k  = ACT relu(as_k*Vt_k + ab_k)
Validated vs the fp64 oracle in numpy: rel err ~ 4.9e-4 of output absmax.
"""

import os

os.environ.setdefault("MYCRO_LOCAL_CACHE", "1")

import numpy as np

import concourse.bacc as bacc
import concourse.bass as bass
import concourse.mybir as mybir
from concourse.tile import TileContext

N_CORES = 8
B, IN, E, L = 8192, 1024, 4096, 8
BC = B // N_CORES          # 1024 batch rows per core
NI = IN // 128             # 8 i-chunks
NE = E // 128              # 32 e-chunks
F16 = mybir.dt.float16
F32 = mybir.dt.float32
NCONST = 24

GROUP = 4                  # e-chunks interleaved in the cascade pipeline
W1ECS = 20                 # mm2 window-1 depth (overlapped under the cascade)
EVICT_DVE_MOD = 8          # every Nth h-eviction runs on DVE, rest on ACT
GP_LAYERS = ()             # GPSIMD TT offload: net loss (shared SBUF port
                           # contention doubles DVE ts cost) — keep empty


_SEQ_ONLY = {
    "InstUnconditionalBranch",
    "InstCall",
    "InstISA",
}


def _legalize_waits(nc):
    """Datapath instructions carry exactly ONE semaphore wait slot in the
    64-byte ISA encoding (walrus errors on more). Engine sequencers execute
    their stream in order, so any extra waits can be hoisted onto single-wait
    NoOps inserted immediately before the capped instruction — semantically
    identical (all waits still complete before the instruction executes).
    For HWDGE DMAs prefer keeping a DMA-queue wait in-descriptor and hoist
    engine-sem waits to the sequencer."""
    import bass_rust

    uid = 0
    for bb in nc.m.functions[0].blocks:
        insts = bb.instructions  # live list
        newlist = []
        for i in insts:
            cls = i.__class__.__name__
            si = i.sync_info
            if cls in _SEQ_ONLY or si is None or len(si.on_wait) <= 1:
                newlist.append(i)
                continue
            waits = list(si.on_wait)
            if cls == "InstDMACopy":
                dmaw = [w for w in waits if w.ant_name.startswith("DMA")]
                keep = dmaw[-1] if dmaw else waits[-1]
            else:
                keep = waits[-1]
            rest = [w for w in waits if w is not keep]
            for w in rest:
                uid += 1
                nop = mybir.InstNoOp(
                    name=f"waitnop-{uid}-{i.name}",
                    engine=i.engine,
                    bass_nofuse=True,
                )
                nop.sync_info = bass_rust.SyncInfo(on_wait=[w], on_update=[])
                newlist.append(nop)
            si.on_wait = [keep]
            newlist.append(i)
        if len(newlist) != len(insts):
            insts[:] = newlist


def build_nc() -> bass.Bass:
    nc = bacc.Bacc()
    AL = mybir.AluOpType
    AF = mybir.ActivationFunctionType

    xTd = nc.declare_dram_parameter("xT", [128, NI, BC], F16, isOutput=False)
    WTd = nc.declare_dram_parameter("WT", [128, NE, NI, 128], F16, isOutput=False)
    W2d = nc.declare_dram_parameter("W2", [128, NE, IN], F16, isOutput=False)
    Cd = nc.declare_dram_parameter("consts", [128, NE, NCONST], F32, isOutput=False)
    Bd = nc.declare_dram_parameter("blast", [128, NI], F32, isOutput=False)
    Od = nc.declare_dram_parameter("outT", [128, NI, BC], F16, isOutput=True)

    with TileContext(nc) as tc:
        with (
            tc.tile_pool(name="persist", bufs=1) as persist,
            tc.tile_pool(name="wtp", bufs=4) as wtp,
            tc.tile_pool(name="w2p", bufs=3) as w2p,
            tc.tile_pool(name="hsbp", bufs=4) as hsbp,
            tc.tile_pool(name="upool", bufs=4) as upool,
            tc.tile_pool(name="vtpool", bufs=3) as vtpool,
            tc.tile_pool(name="hbpool", bufs=3) as hbpool,
            tc.tile_pool(name="ppool", bufs=2) as ppool,
            tc.tile_pool(name="outp", bufs=2) as outp,
            tc.tile_pool(name="psum_h", bufs=3, space="PSUM") as psum_h,
            tc.tile_pool(name="psum_o", bufs=2, space="PSUM") as psum_o,
        ):
            consts_sb = persist.tile([128, NE, NCONST], F32)
            nc.sync.dma_start(out=consts_sb, in_=Cd[:, :, :])
            blast_sb = persist.tile([128, NI], F32)
            nc.sync.dma_start(out=blast_sb, in_=Bd[:, :])
            x_sb = persist.tile([128, NI, BC], F16)
            # x split across two DMA queues with the first weight tile
            # leading the scalar queue, so mm1 can start within ~4us
            pre_wt = {}
            pre_wt[0] = wtp.tile([128, NI, 128], F16, tag="wt", name="wt_pre0")
            nc.scalar.dma_start(out=pre_wt[0], in_=WTd[:, 0, :, :])
            for i in range(NI):
                q = nc.sync if i % 2 == 0 else nc.scalar
                q.dma_start(out=x_sb[:, i, :], in_=xTd[:, i, :])
            for ec in range(1, GROUP):
                wt = wtp.tile([128, NI, 128], F16, tag="wt")
                nc.sync.dma_start(out=wt, in_=WTd[:, ec, :, :])
                pre_wt[ec] = wt
            z_sb = persist.tile([128, NE, BC], F16)
            o_acc = persist.tile([128, NI, BC], F16)

            def c_ap(ec, col):
                return consts_sb[:, ec, col : col + 1]

            w2_tiles = {}

            def load_w2(ic, ec_lo, ec_hi):
                t = w2p.tile(
                    [128, W1ECS, 128], F16, tag="w2", name=f"w2_{ic}_{ec_lo}"
                )
                nc.sync.dma_start(
                    out=t[:, : ec_hi - ec_lo, :],
                    in_=W2d[:, ec_lo:ec_hi, ic * 128 : (ic + 1) * 128],
                )
                w2_tiles[(ic, ec_lo)] = t

            def emit_mm2_group(ic, hf, ec_lo, ec_hi, into_acc):
                w2t = w2_tiles[(ic, ec_lo)]
                ops = psum_o.tile(
                    [128, 512], F32, tag="o", name=f"o_{ic}_{hf}_{ec_lo}"
                )
                for ec in range(ec_lo, ec_hi):
                    nc.tensor.matmul(
                        ops,
                        w2t[:, ec - ec_lo, :],
                        z_sb[:, ec, hf * 512 : (hf + 1) * 512],
                        start=(ec == ec_lo),
                        stop=(ec == ec_hi - 1),
                    )
                bsl = hf * 512
                if into_acc:
                    # bias_last folded here; fp16 partial staging
                    nc.scalar.activation(
                        out=o_acc[:, ic, bsl : bsl + 512],
                        in_=ops,
                        func=AF.Identity,
                        bias=blast_sb[:, ic : ic + 1],
                        scale=1.0,
                    )
                else:
                    osb = outp.tile([128, 512], F16, tag="osb")
                    nc.vector.tensor_tensor(
                        out=osb, in0=o_acc[:, ic, bsl : bsl + 512], in1=ops,
                        op=AL.add,
                    )
                    nc.scalar.dma_start(
                        out=Od[:, ic, bsl : bsl + 512], in_=osb
                    )

            # window-1 whole-ic units spread over phase-A tail groups,
            # emitted AFTER each group's mm1 so they don't starve the cascade
            w1_sched = {20: range(0, 3), 24: range(3, 5), 28: range(5, 8)}

            # ---------------- Phase A: mm1 + cascade ----------------
            for g0 in range(0, NE, GROUP):
                ecs = list(range(g0, g0 + GROUP))
                h_ps = {}
                for ec in ecs:
                    if ec in pre_wt:
                        wt = pre_wt[ec]
                    else:
                        wt = wtp.tile([128, NI, 128], F16, tag="wt")
                        nc.sync.dma_start(out=wt, in_=WTd[:, ec, :, :])
                    hp = psum_h.tile([128, BC], F32, tag="h")
                    for i in range(NI):
                        lhsT = wt[:, i, :]
                        for hf in range(2):
                            nc.tensor.matmul(
                                hp[:, hf * 512 : (hf + 1) * 512],
                                lhsT,
                                x_sb[:, i, hf * 512 : (hf + 1) * 512],
                                start=(i == 0),
                                stop=(i == NI - 1),
                            )
                    h_ps[ec] = hp

                for ic in w1_sched.get(g0, ()):
                    load_w2(ic, 0, W1ECS)
                    for hf in range(2):
                        emit_mm2_group(ic, hf, 0, W1ECS, into_acc=True)

                pairs = [(ecs[0], ecs[1]), (ecs[2], ecs[3])]
                hsb = {}
                for pi, (ea, eb) in enumerate(pairs):
                    t = hsbp.tile([128, 2, BC], F16, tag="hsb", name=f"hsb_{ea}")
                    for j, ec in ((0, ea), (1, eb)):
                        if ec % EVICT_DVE_MOD == 0:
                            nc.vector.tensor_copy(out=t[:, j, :], in_=h_ps[ec])
                        else:
                            nc.scalar.copy(out=t[:, j, :], in_=h_ps[ec])
                    hsb[pi] = t

                # ---- cascade, layer-major across the 2 pairs ----
                u_cur = {}
                # D-layer 1: P_1 then Uo_1 = relu(P_1) * as_2
                for pi, (ea, eb) in enumerate(pairs):
                    pt = ppool.tile([128, 2, BC], F16, tag="p", name=f"p1_{ea}")
                    for j, ec in ((0, ea), (1, eb)):
                        nc.vector.tensor_scalar(
                            pt[:, j, :], hsb[pi][:, j, :],
                            c_ap(ec, 8), c_ap(ec, 16), AL.mult, AL.add,
                        )
                    ut = upool.tile([128, 2, BC], F16, tag="u", name=f"u1_{ea}")
                    for j, ec in ((0, ea), (1, eb)):
                        nc.vector.tensor_scalar(
                            ut[:, j, :], pt[:, j, :],
                            0.0, c_ap(ec, 9), AL.max, AL.mult,
                        )
                    u_cur[pi] = ut
                # D-layer 2: hb_2, P_2 = hb_2 + Uo_1, Uo_2 = relu(P_2)
                for pi, (ea, eb) in enumerate(pairs):
                    hbt = hbpool.tile([128, 2, BC], F16, tag="hb", name=f"hb2_{ea}")
                    for j, ec in ((0, ea), (1, eb)):
                        nc.vector.tensor_scalar(
                            hbt[:, j, :], hsb[pi][:, j, :],
                            c_ap(ec, 6), c_ap(ec, 17), AL.mult, AL.add,
                        )
                    pt = ppool.tile([128, 2, BC], F16, tag="p", name=f"p2_{ea}")
                    nc.vector.tensor_tensor(
                        out=pt[:, :, :], in0=hbt[:, :, :], in1=u_cur[pi][:, :, :],
                        op=AL.add,
                    )
                    ut = upool.tile([128, 2, BC], F16, tag="u", name=f"u2_{ea}")
                    nc.vector.tensor_scalar(
                        ut[:, :, :], pt[:, :, :], 0.0, None, AL.max,
                    )
                    u_cur[pi] = ut
                # A-layers 3..8
                for k in range(3, L + 1):
                    for pi, (ea, eb) in enumerate(pairs):
                        hbt = hbpool.tile(
                            [128, 2, BC], F16, tag="hb", name=f"hb{k}_{ea}"
                        )
                        for j, ec in ((0, ea), (1, eb)):
                            nc.vector.tensor_scalar(
                                hbt[:, j, :], hsb[pi][:, j, :],
                                c_ap(ec, k - 3), None, AL.mult,
                            )
                        vt = vtpool.tile(
                            [128, 2, BC], F16, tag="vt", name=f"vt{k}_{ea}"
                        )
                        tt_eng = nc.gpsimd if k in GP_LAYERS else nc.vector
                        tt_eng.tensor_tensor(
                            out=vt[:, :, :], in0=hbt[:, :, :],
                            in1=u_cur[pi][:, :, :], op=AL.add,
                        )
                        if k < L:
                            ut = upool.tile(
                                [128, 2, BC], F16, tag="u", name=f"u{k}_{ea}"
                            )
                        for j, ec in ((0, ea), (1, eb)):
                            dst = z_sb[:, ec, :] if k == L else ut[:, j, :]
                            nc.scalar.activation(
                                out=dst,
                                in_=vt[:, j, :],
                                func=AF.Relu,
                                bias=c_ap(ec, 16 + k - 1),
                                scale=c_ap(ec, 8 + k - 1),
                            )
                        if k < L:
                            u_cur[pi] = ut

            # ---------------- Phase B: mm2 window-2 + combine ----------------
            for ic in range(NI):
                load_w2(ic, W1ECS, NE)
                for hf in range(2):
                    emit_mm2_group(ic, hf, W1ECS, NE, into_acc=False)

    nc.compile()
    return nc


def _prep_inputs(x, W, biases, bias_last, alpha, beta):
    """Host-side shard/relayout/constant precompute. Returns per-core in_maps."""
    x = np.asarray(x, np.float32)
    W = np.asarray(W, np.float32)
    biases = np.asarray(biases, np.float32)
    bias_last = np.asarray(bias_last, np.float32)
    alpha = np.asarray(alpha, np.float32)
    beta = np.asarray(beta, np.float32)

    sgn = lambda a: np.where(a >= 0, 1.0, -1.0).astype(np.float32)
    Bk = beta * biases[:L]                      # (8, E)
    sigma = np.ones((L + 1, E), np.float32)     # sigma[k], k=1..8
    for k in range(1, L):
        sigma[k + 1] = sgn(alpha[k - 1])
    s_last = sgn(alpha[L - 1])

    ss = np.zeros((L + 1, E), np.float32)       # ss_k, k=1..7
    as_ = np.zeros((L + 1, E), np.float32)      # as_k, k=1..8
    ab = np.zeros((L + 1, E), np.float32)
    for k in range(1, L):
        ss[k] = sigma[k + 1] * beta[k]
    for k in range(1, L + 1):
        sck = sigma[k] * np.abs(alpha[k - 1])
        if k == 1:
            sck = sck * beta[0]                 # layer-1 reads h directly
        as_[k] = sck
        ab[k] = np.abs(alpha[k - 1]) * Bk[k - 1]

    consts = np.zeros((E, NCONST), np.float32)
    for k in range(2, L):                       # ss_2..ss_7 -> cols 0..5
        consts[:, k - 2] = ss[k]
    consts[:, 6] = as_[2] * ss[1]               # hb_2 scale
    for k in range(1, L + 1):                   # as cols 8..15, ab cols 16..23
        consts[:, 7 + k] = as_[k]
        consts[:, 15 + k] = ab[k]
    consts_t = np.ascontiguousarray(
        consts.reshape(NE, 128, NCONST).transpose(1, 0, 2)
    )

    WT_t = np.ascontiguousarray(
        W.T.reshape(NI, 128, NE, 128).transpose(1, 2, 0, 3).astype(np.float16)
    )
    W2 = W * s_last[:, None]
    W2_t = np.ascontiguousarray(
        W2.reshape(NE, 128, IN).transpose(1, 0, 2).astype(np.float16)
    )
    blast_t = np.ascontiguousarray(bias_last.reshape(NI, 128).T)

    in_maps = []
    for c in range(N_CORES):
        xc = x[c * BC : (c + 1) * BC]           # (BC, IN)
        xT = np.ascontiguousarray(
            xc.T.reshape(NI, 128, BC).transpose(1, 0, 2).astype(np.float16)
        )
        in_maps.append(
            {
                "xT": xT,
                "WT": WT_t,
                "W2": W2_t,
                "consts": consts_t,
                "blast": blast_t,
            }
        )
    return in_maps


_NC_CACHE = None


def _install_ntff_hook():
    """The agent image's antenv lacks axon_hooks; rebuild it from the boot
    helper so run_bass_kernel_spmd(trace=True) can capture NTFF profiles."""
    import sys
    import types

    if "antenv.axon_hooks" in sys.modules:
        return
    try:
        from trn_agent_boot.trn_boot import _ntff_profile_via_ctypes

        hook = _ntff_profile_via_ctypes("/opt/axon/libaxon_pjrt.so")
    except Exception:
        hook = None
    m = types.ModuleType("antenv.axon_hooks")
    m.get_axon_ntff_profile_hook = lambda: hook
    m.set_axon_ntff_profile_hook = lambda h: None
    sys.modules["antenv.axon_hooks"] = m


def run(inputs: dict, trace: bool = False):
    """Returns (out, BassKernelResults)."""
    global _NC_CACHE
    from concourse.bass_utils import run_bass_kernel_spmd

    if trace:
        _install_ntff_hook()

    if _NC_CACHE is None:
        _NC_CACHE = build_nc()
    nc = _NC_CACHE
    in_maps = _prep_inputs(**inputs)
    res = run_bass_kernel_spmd(nc, in_maps, list(range(N_CORES)), trace=trace)
    out = np.empty((B, IN), np.float32)
    for c in range(N_CORES):
        oc = np.asarray(res.results[c]["outT"]).astype(np.float32)
        out_core = oc.transpose(1, 0, 2).reshape(IN, BC) # (IN, BC) = outT
        out[c * BC : (c + 1) * BC] = out_core.T
    return out, res


def kernel(x, W, biases, bias_last, alpha, beta) -> np.ndarray:
    out, _ = run(
        dict(x=x, W=W, biases=biases, bias_last=bias_last, alpha=alpha, beta=beta)
    )
    return out


# revision 26
# speedup vs baseline: 1.0314x; 1.0038x over previous
"""Trainium2 Bass kernel for nn_CascadeGradNetOURS (dense_mlp).

Math (reference):
    h = x @ W.T                       # (B, E), shared by all layers
    z = beta[0] * (h + b[0])
    for i in 0..6:
        z = beta[i+1]*(h + b[i+1]) + alpha[i]*relu(z)
    z = alpha[7] * relu(z)
    out = z @ W + bias_last           # (B, IN)

Device formulation (per core, batch-sharded 1024 rows, transposed layout
hT[e, b] so per-layer alpha/beta/bias become per-PARTITION scalars).

Baseline recurrence (sign-deferred, verified):
    Vt_1 = h;  U_k = relu(as_k*Vt_k + ab_k);  Vt_{k+1} = ss_k*h + U_k
    z = U_8 (|alpha_7| & sign folded into W2 rows); out = z @ W2 + blast.

Op grouping here avoids the 1x-rate SCALAR_TENSOR_TENSOR entirely:
all elementwise work is 2x-rate tensor_scalar (2 ALU slots, per-partition
AP scalars), 2x tensor_tensor adds batched over ec-pairs, plus one
ACTIVATE per layer for 6 of the 8 relus:
    D-layer 1:  P_1  = ts(h, as_1, ab_1; mult, add)
                Uo_1 = ts(P_1, 0.0, as_2; max, mult)
    D-layer 2:  hb_2 = ts(h, as_2*ss_1, ab_2; mult, add)
                P_2  = TT_add(hb_2, Uo_1)        [ec-pair batched]
                Uo_2 = ts(P_2, 0.0; max)         [ec-pair batched]
    A-layers k=3..8:
                hb_k = ts(h, ss_{k-1}; mult)
                Vt_k = TT_add(hb_k, U_{k-1})     [ec-pair batched]
                U_k  = ACT relu(as_k*Vt_k + ab_k)
Validated vs the fp64 oracle in numpy: rel err ~ 4.9e-4 of output absmax.
"""

import os

os.environ.setdefault("MYCRO_LOCAL_CACHE", "1")

import numpy as np

import concourse.bacc as bacc
import concourse.bass as bass
import concourse.mybir as mybir
from concourse.tile import TileContext

N_CORES = 8
B, IN, E, L = 8192, 1024, 4096, 8
BC = B // N_CORES          # 1024 batch rows per core
NI = IN // 128             # 8 i-chunks
NE = E // 128              # 32 e-chunks
F16 = mybir.dt.float16
F32 = mybir.dt.float32
NCONST = 24

GROUP = 4                  # e-chunks interleaved in the cascade pipeline
W1ECS = 20                 # mm2 window-1 depth (overlapped under the cascade)
EVICT_DVE_MOD = 8          # every Nth h-eviction runs on DVE, rest on ACT
GP_LAYERS = ()             # GPSIMD TT offload: net loss (shared SBUF port
                           # contention doubles DVE ts cost) — keep empty


_SEQ_ONLY = {
    "InstUnconditionalBranch",
    "InstCall",
    "InstISA",
}


def _legalize_waits(nc):
    """Datapath instructions carry exactly ONE semaphore wait slot in the
    64-byte ISA encoding (walrus errors on more). Engine sequencers execute
    their stream in order, so any extra waits can be hoisted onto single-wait
    NoOps inserted immediately before the capped instruction — semantically
    identical (all waits still complete before the instruction executes).
    For HWDGE DMAs prefer keeping a DMA-queue wait in-descriptor and hoist
    engine-sem waits to the sequencer."""
    import bass_rust

    uid = 0
    for bb in nc.m.functions[0].blocks:
        insts = bb.instructions  # live list
        newlist = []
        for i in insts:
            cls = i.__class__.__name__
            si = i.sync_info
            if cls in _SEQ_ONLY or si is None or len(si.on_wait) <= 1:
                newlist.append(i)
                continue
            waits = list(si.on_wait)
            if cls == "InstDMACopy":
                dmaw = [w for w in waits if w.ant_name.startswith("DMA")]
                keep = dmaw[-1] if dmaw else waits[-1]
            else:
                keep = waits[-1]
            rest = [w for w in waits if w is not keep]
            for w in rest:
                uid += 1
                nop = mybir.InstNoOp(
                    name=f"waitnop-{uid}-{i.name}",
                    engine=i.engine,
                    bass_nofuse=True,
                )
                nop.sync_info = bass_rust.SyncInfo(on_wait=[w], on_update=[])
                newlist.append(nop)
            si.on_wait = [keep]
            newlist.append(i)
        if len(newlist) != len(insts):
            insts[:] = newlist


def build_nc() -> bass.Bass:
    nc = bacc.Bacc()
    AL = mybir.AluOpType
    AF = mybir.ActivationFunctionType

    xTd = nc.declare_dram_parameter("xT", [128, NI, BC], F16, isOutput=False)
    WTd = nc.declare_dram_parameter("WT", [128, NE, NI, 128], F16, isOutput=False)
    W2d = nc.declare_dram_parameter("W2", [128, NE, IN], F16, isOutput=False)
    Cd = nc.declare_dram_parameter("consts", [128, NE, NCONST], F32, isOutput=False)
    Bd = nc.declare_dram_parameter("blast", [128, NI], F32, isOutput=False)
    Od = nc.declare_dram_parameter("outT", [128, NI, BC], F16, isOutput=True)

    with TileContext(nc) as tc:
        with (
            tc.tile_pool(name="persist", bufs=1) as persist,
            tc.tile_pool(name="wtp", bufs=4) as wtp,
            tc.tile_pool(name="w2p", bufs=3) as w2p,
            tc.tile_pool(name="hsbp", bufs=4) as hsbp,
            tc.tile_pool(name="upool", bufs=4) as upool,
            tc.tile_pool(name="vtpool", bufs=3) as vtpool,
            tc.tile_pool(name="hbpool", bufs=3) as hbpool,
            tc.tile_pool(name="ppool", bufs=2) as ppool,
            tc.tile_pool(name="outp", bufs=2) as outp,
            tc.tile_pool(name="psum_h", bufs=3, space="PSUM") as psum_h,
            tc.tile_pool(name="psum_o", bufs=2, space="PSUM") as psum_o,
        ):
            consts_sb = persist.tile([128, NE, NCONST], F32)
            nc.sync.dma_start(out=consts_sb, in_=Cd[:, :, :])
            blast_sb = persist.tile([128, NI], F32)
            nc.sync.dma_start(out=blast_sb, in_=Bd[:, :])
            x_sb = persist.tile([128, NI, BC], F16)
            # x split across two DMA queues with the first weight tile
            # leading the scalar queue, so mm1 can start within ~4us
            pre_wt = {}
            pre_wt[0] = wtp.tile([128, NI, 128], F16, tag="wt", name="wt_pre0")
            nc.scalar.dma_start(out=pre_wt[0][:, 0:2, :], in_=WTd[:, 0, 0:2, :])
            nc.sync.dma_start(out=x_sb[:, 0, 0:512], in_=xTd[:, 0, 0:512])
            nc.scalar.dma_start(out=pre_wt[0][:, 2:, :], in_=WTd[:, 0, 2:, :])
            nc.sync.dma_start(out=x_sb[:, 0, 512:], in_=xTd[:, 0, 512:])
            for i in range(1, NI):
                q = nc.sync if i % 2 == 0 else nc.scalar
                q.dma_start(out=x_sb[:, i, :], in_=xTd[:, i, :])
            for ec in range(1, GROUP):
                wt = wtp.tile([128, NI, 128], F16, tag="wt")
                nc.sync.dma_start(out=wt, in_=WTd[:, ec, :, :])
                pre_wt[ec] = wt
            z_sb = persist.tile([128, NE, BC], F16)
            o_acc = persist.tile([128, NI, BC], F16)

            def c_ap(ec, col):
                return consts_sb[:, ec, col : col + 1]

            w2_tiles = {}

            def load_w2(ic, ec_lo, ec_hi):
                t = w2p.tile(
                    [128, W1ECS, 128], F16, tag="w2", name=f"w2_{ic}_{ec_lo}"
                )
                nc.sync.dma_start(
                    out=t[:, : ec_hi - ec_lo, :],
                    in_=W2d[:, ec_lo:ec_hi, ic * 128 : (ic + 1) * 128],
                )
                w2_tiles[(ic, ec_lo)] = t

            def emit_mm2_group(ic, hf, ec_lo, ec_hi, into_acc):
                w2t = w2_tiles[(ic, ec_lo)]
                ops = psum_o.tile(
                    [128, 512], F32, tag="o", name=f"o_{ic}_{hf}_{ec_lo}"
                )
                bsl = hf * 512
                for ec in range(ec_lo, ec_hi):
                    nc.tensor.matmul(
                        ops,
                        w2t[:, ec - ec_lo, :],
                        z_sb[:, ec, hf * 512 : (hf + 1) * 512],
                        start=(ec == ec_lo),
                        stop=(ec == ec_hi - 1),
                    )
                if into_acc:
                    # bias_last folded here; fp16 partial staging
                    nc.scalar.activation(
                        out=o_acc[:, ic, bsl : bsl + 512],
                        in_=ops,
                        func=AF.Identity,
                        bias=blast_sb[:, ic : ic + 1],
                        scale=1.0,
                    )
                else:
                    osb = outp.tile([128, 512], F16, tag="osb")
                    nc.vector.tensor_tensor(
                        out=osb, in0=o_acc[:, ic, bsl : bsl + 512], in1=ops,
                        op=AL.add,
                    )
                    nc.scalar.dma_start(
                        out=Od[:, ic, bsl : bsl + 512], in_=osb
                    )

            # window-1 whole-ic units spread over phase-A tail groups,
            # emitted AFTER each group's mm1 so they don't starve the cascade
            w1_sched = {20: range(0, 3), 24: range(3, 5), 28: range(5, 8)}

            # ---------------- Phase A: mm1 + cascade ----------------
            for g0 in range(0, NE, GROUP):
                ecs = list(range(g0, g0 + GROUP))
                h_ps = {}
                for ec in ecs:
                    if ec in pre_wt:
                        wt = pre_wt[ec]
                    else:
                        wt = wtp.tile([128, NI, 128], F16, tag="wt")
                        nc.sync.dma_start(out=wt, in_=WTd[:, ec, :, :])
                    hp = psum_h.tile([128, BC], F32, tag="h")
                    for i in range(NI):
                        lhsT = wt[:, i, :]
                        for hf in range(2):
                            nc.tensor.matmul(
                                hp[:, hf * 512 : (hf + 1) * 512],
                                lhsT,
                                x_sb[:, i, hf * 512 : (hf + 1) * 512],
                                start=(i == 0),
                                stop=(i == NI - 1),
                            )
                    h_ps[ec] = hp

                for ic in w1_sched.get(g0, ()):
                    load_w2(ic, 0, W1ECS)
                    for hf in range(2):
                        emit_mm2_group(ic, hf, 0, W1ECS, into_acc=True)

                pairs = [(ecs[0], ecs[1]), (ecs[2], ecs[3])]
                hsb = {}
                for pi, (ea, eb) in enumerate(pairs):
                    t = hsbp.tile([128, 2, BC], F16, tag="hsb", name=f"hsb_{ea}")
                    for j, ec in ((0, ea), (1, eb)):
                        if ec % EVICT_DVE_MOD == 0:
                            nc.vector.tensor_copy(out=t[:, j, :], in_=h_ps[ec])
                        else:
                            nc.scalar.copy(out=t[:, j, :], in_=h_ps[ec])
                    hsb[pi] = t

                # ---- cascade, layer-major across the 2 pairs ----
                u_cur = {}
                # D-layer 1: P_1 then Uo_1 = relu(P_1) * as_2
                for pi, (ea, eb) in enumerate(pairs):
                    pt = ppool.tile([128, 2, BC], F16, tag="p", name=f"p1_{ea}")
                    for j, ec in ((0, ea), (1, eb)):
                        nc.vector.tensor_scalar(
                            pt[:, j, :], hsb[pi][:, j, :],
                            c_ap(ec, 8), c_ap(ec, 16), AL.mult, AL.add,
                        )
                    ut = upool.tile([128, 2, BC], F16, tag="u", name=f"u1_{ea}")
                    for j, ec in ((0, ea), (1, eb)):
                        nc.vector.tensor_scalar(
                            ut[:, j, :], pt[:, j, :],
                            0.0, c_ap(ec, 9), AL.max, AL.mult,
                        )
                    u_cur[pi] = ut
                # D-layer 2: hb_2, P_2 = hb_2 + Uo_1, Uo_2 = relu(P_2)
                for pi, (ea, eb) in enumerate(pairs):
                    hbt = hbpool.tile([128, 2, BC], F16, tag="hb", name=f"hb2_{ea}")
                    for j, ec in ((0, ea), (1, eb)):
                        nc.vector.tensor_scalar(
                            hbt[:, j, :], hsb[pi][:, j, :],
                            c_ap(ec, 6), c_ap(ec, 17), AL.mult, AL.add,
                        )
                    pt = ppool.tile([128, 2, BC], F16, tag="p", name=f"p2_{ea}")
                    nc.vector.tensor_tensor(
                        out=pt[:, :, :], in0=hbt[:, :, :], in1=u_cur[pi][:, :, :],
                        op=AL.add,
                    )
                    ut = upool.tile([128, 2, BC], F16, tag="u", name=f"u2_{ea}")
                    nc.vector.tensor_scalar(
                        ut[:, :, :], pt[:, :, :], 0.0, None, AL.max,
                    )
                    u_cur[pi] = ut
                # A-layers 3..8
                for k in range(3, L + 1):
                    for pi, (ea, eb) in enumerate(pairs):
                        hbt = hbpool.tile(
                            [128, 2, BC], F16, tag="hb", name=f"hb{k}_{ea}"
                        )
                        for j, ec in ((0, ea), (1, eb)):
                            nc.vector.tensor_scalar(
                                hbt[:, j, :], hsb[pi][:, j, :],
                                c_ap(ec, k - 3), None, AL.mult,
                            )
                        vt = vtpool.tile(
                            [128, 2, BC], F16, tag="vt", name=f"vt{k}_{ea}"
                        )
                        tt_eng = nc.gpsimd if k in GP_LAYERS else nc.vector
                        tt_eng.tensor_tensor(
                            out=vt[:, :, :], in0=hbt[:, :, :],
                            in1=u_cur[pi][:, :, :], op=AL.add,
                        )
                        if k < L:
                            ut = upool.tile(
                                [128, 2, BC], F16, tag="u", name=f"u{k}_{ea}"
                            )
                        for j, ec in ((0, ea), (1, eb)):
                            dst = z_sb[:, ec, :] if k == L else ut[:, j, :]
                            nc.scalar.activation(
                                out=dst,
                                in_=vt[:, j, :],
                                func=AF.Relu,
                                bias=c_ap(ec, 16 + k - 1),
                                scale=c_ap(ec, 8 + k - 1),
                            )
                        if k < L:
                            u_cur[pi] = ut

            # ---------------- Phase B: mm2 window-2 + combine ----------------
            for ic in range(NI):
                load_w2(ic, W1ECS, NE)
                for hf in range(2):
                    emit_mm2_group(ic, hf, W1ECS, NE, into_acc=False)

    nc.compile()
    return nc


def _prep_inputs(x, W, biases, bias_last, alpha, beta):
    """Host-side shard/relayout/constant precompute. Returns per-core in_maps."""
    x = np.asarray(x, np.float32)
    W = np.asarray(W, np.float32)
    biases = np.asarray(biases, np.float32)
    bias_last = np.asarray(bias_last, np.float32)
    alpha = np.asarray(alpha, np.float32)
    beta = np.asarray(beta, np.float32)

    sgn = lambda a: np.where(a >= 0, 1.0, -1.0).astype(np.float32)
    Bk = beta * biases[:L]                      # (8, E)
    sigma = np.ones((L + 1, E), np.float32)     # sigma[k], k=1..8
    for k in range(1, L):
        sigma[k + 1] = sgn(alpha[k - 1])
    s_last = sgn(alpha[L - 1])

    ss = np.zeros((L + 1, E), np.float32)       # ss_k, k=1..7
    as_ = np.zeros((L + 1, E), np.float32)      # as_k, k=1..8
    ab = np.zeros((L + 1, E), np.float32)
    for k in range(1, L):
        ss[k] = sigma[k + 1] * beta[k]
    for k in range(1, L + 1):
        sck = sigma[k] * np.abs(alpha[k - 1])
        if k == 1:
            sck = sck * beta[0]                 # layer-1 reads h directly
        as_[k] = sck
        ab[k] = np.abs(alpha[k - 1]) * Bk[k - 1]

    consts = np.zeros((E, NCONST), np.float32)
    for k in range(2, L):                       # ss_2..ss_7 -> cols 0..5
        consts[:, k - 2] = ss[k]
    consts[:, 6] = as_[2] * ss[1]               # hb_2 scale
    for k in range(1, L + 1):                   # as cols 8..15, ab cols 16..23
        consts[:, 7 + k] = as_[k]
        consts[:, 15 + k] = ab[k]
    consts_t = np.ascontiguousarray(
        consts.reshape(NE, 128, NCONST).transpose(1, 0, 2)
    )

    WT_t = np.ascontiguousarray(
        W.T.reshape(NI, 128, NE, 128).transpose(1, 2, 0, 3).astype(np.float16)
    )
    W2 = W * s_last[:, None]
    W2_t = np.ascontiguousarray(
        W2.reshape(NE, 128, IN).transpose(1, 0, 2).astype(np.float16)
    )
    blast_t = np.ascontiguousarray(bias_last.reshape(NI, 128).T)

    in_maps = []
    for c in range(N_CORES):
        xc = x[c * BC : (c + 1) * BC]           # (BC, IN)
        xT = np.ascontiguousarray(
            xc.T.reshape(NI, 128, BC).transpose(1, 0, 2).astype(np.float16)
        )
        in_maps.append(
            {
                "xT": xT,
                "WT": WT_t,
                "W2": W2_t,
                "consts": consts_t,
                "blast": blast_t,
            }
        )
    return in_maps


_NC_CACHE = None


def _install_ntff_hook():
    """The agent image's antenv lacks axon_hooks; rebuild it from the boot
    helper so run_bass_kernel_spmd(trace=True) can capture NTFF profiles."""
    import sys
    import types

    if "antenv.axon_hooks" in sys.modules:
        return
    try:
        from trn_agent_boot.trn_boot import _ntff_profile_via_ctypes

        hook = _ntff_profile_via_ctypes("/opt/axon/libaxon_pjrt.so")
    except Exception:
        hook = None
    m = types.ModuleType("antenv.axon_hooks")
    m.get_axon_ntff_profile_hook = lambda: hook
    m.set_axon_ntff_profile_hook = lambda h: None
    sys.modules["antenv.axon_hooks"] = m


def run(inputs: dict, trace: bool = False):
    """Returns (out, BassKernelResults)."""
    global _NC_CACHE
    from concourse.bass_utils import run_bass_kernel_spmd

    if trace:
        _install_ntff_hook()

    if _NC_CACHE is None:
        _NC_CACHE = build_nc()
    nc = _NC_CACHE
    in_maps = _prep_inputs(**inputs)
    res = run_bass_kernel_spmd(nc, in_maps, list(range(N_CORES)), trace=trace)
    out = np.empty((B, IN), np.float32)
    for c in range(N_CORES):
        oc = np.asarray(res.results[c]["outT"]).astype(np.float32)
        out_core = oc.transpose(1, 0, 2).reshape(IN, BC) # (IN, BC) = outT
        out[c * BC : (c + 1) * BC] = out_core.T
    return out, res


def kernel(x, W, biases, bias_last, alpha, beta) -> np.ndarray:
    out, _ = run(
        dict(x=x, W=W, biases=biases, bias_last=bias_last, alpha=alpha, beta=beta)
    )
    return out
